# revision 21
# baseline (speedup 1.0000x reference)
"""Trainium2 Bass kernel for batched attention (B=8, Lq=Lk=2048, D=Dv=128).

Sharding: pure data parallel — batch element b runs on NeuronCore b.
Per-core algorithm (all big matmuls bf16 with fp32 PSUM accumulation):

  qT = Wq^T @ xq^T        [d, Lq]   (PE transposes raw tiles)
  kT = Wk^T @ xk^T        [d, Lk]
  v  = xv @ Wv            [Lk, dv]
  for each k-tile j (16 tiles of 128):
      sT_j = kT_j^T @ qT  [128k, Lq]   (scores, TRANSPOSED: k on partitions)
      aT_j = exp(sT_j * scale + mask_bias)   (ACT, psum->sbuf, bf16)
      S   += aT_j                 (running col-sum: DVE cols 0:640, Pool 640:)
      oT  += v_j^T @ aT_j [dv, Lq]      (PSUM accumulate across j)
  denT[:, t] = S_t^T @ ones             (per-q softmax denominators)
  out_t = (oT_t)^T * recip(denT_t)

Schedule: inputs stream in as quarters over all three DMA queues
(sync/scalar HWDGE fp32 + gpsimd SWDGE casting to bf16) dispatched at
program start in deadline order; prep (transpose+proj) for everything
past k.Q0/q.h0 is interleaved into the 32-iteration main loop; PE
fillers keep the HAM clock gate open until the first real transpose;
epilogues are pipelined and the final half double-emits lagged oT
matmuls so only 2 k-tiles remain to flush at the end.
"""

import sys

sys.path.insert(0, "/opt/trn_rl_repo")

import numpy as np

import concourse.bass as bass
import concourse.mybir as mybir
import concourse.tile as tile
from concourse import bacc
from concourse.bass_utils import run_bass_kernel_spmd
from concourse.masks import make_identity

P = 128
L = 2048
D = 128
T = L // P  # 16 tiles
F32 = mybir.dt.float32
I32 = mybir.dt.int32
BF16 = mybir.dt.bfloat16
SCALE = 1.0 / float(np.sqrt(128.0))
N_CORES = 8
SSPLIT = 640  # S-accumulation column split: DVE [0:640), Pool [640:)

ADD = mybir.AluOpType.add
MULT = mybir.AluOpType.mult
EXP = mybir.ActivationFunctionType.Exp
COPY_FN = mybir.ActivationFunctionType.Copy


def build():
    nc = bacc.Bacc("TRN2", target_bir_lowering=False, debug=False)

    q_ext = nc.declare_dram_parameter("query", [L, D], F32, isOutput=False)
    k_ext = nc.declare_dram_parameter("key", [L, D], F32, isOutput=False)
    v_ext = nc.declare_dram_parameter("value", [L, D], F32, isOutput=False)
    wq_ext = nc.declare_dram_parameter("Wq", [D, D], F32, isOutput=False)
    wk_ext = nc.declare_dram_parameter("Wk", [D, D], F32, isOutput=False)
    wv_ext = nc.declare_dram_parameter("Wv", [D, D], F32, isOutput=False)
    m_ext = nc.declare_dram_parameter("mask", [1, L], I32, isOutput=False)
    out_ext = nc.declare_dram_parameter("out", [L, D], BF16, isOutput=True)

    with tile.TileContext(nc) as tc:
        with (
            tc.tile_pool(name="const", bufs=1) as const,
            tc.tile_pool(name="big", bufs=1) as big,
            tc.tile_pool(name="xin", bufs=1) as xin,
            tc.tile_pool(name="xt", bufs=4) as xt,
            tc.tile_pool(name="att", bufs=16) as att,
            # 2 x [128,1024]f32 slots = 4 banks (scores rotation)
            tc.tile_pool(name="ps", bufs=2, space="PSUM") as ps,
            # 2 x [128,512]f32 slots = 2 banks (prep/den/out-transpose)
            tc.tile_pool(name="psp", bufs=2, space="PSUM") as psp,
            # 1 x [128,1024]f32 slot = 2 banks (oT accumulator + warmup)
            tc.tile_pool(name="ps_o", bufs=1, space="PSUM") as ps_o,
        ):
            # ---- gpsimd: warm tile first (unblocks PE), then DMAs ----
            warm = const.tile([P, 512], BF16, tag="warm")
            nc.gpsimd.memset(warm[:], 0.125)

            # identities first on Pool: the first transpose needs ident_f
            ident_f = const.tile([P, P], F32, tag="identf")
            make_identity(nc, ident_f[:])
            ident = const.tile([P, P], BF16, tag="ident")
            nc.gpsimd.tensor_copy(out=ident[:], in_=ident_f[:])

            w_bf = {}
            w_f32 = {}

            # mask bias: element k=(16*p+t) -> [p, t]; bias = (m-1)*1e4
            mask_i = const.tile([P, T], I32, tag="maski")
            nc.gpsimd.dma_start(
                mask_i[:], m_ext[:].rearrange("o (p t) -> p (o t)", p=P)
            )
            wv_b = const.tile([P, D], BF16, tag="w_Wv")
            nc.gpsimd.dma_start(wv_b[:], wv_ext[:])

            # v: cast fp32->bf16 into scratch DRAM (SWDGE), then XBAR
            # DMA-transpose the whole tensor into SBUF as vT [dv, Lk]
            vbf_dram = nc.dram_tensor("vbf_scratch", [L, D], BF16, kind="Internal")
            nc.gpsimd.dma_start(vbf_dram[:], v_ext[:])

            # ---- input quarters: [128, 4, 128], row = 16*p + t ----
            srcs = {
                "q": q_ext[:].rearrange("(p t) d -> p t d", p=P),
                "k": k_ext[:].rearrange("(p t) d -> p t d", p=P),
                "v": v_ext[:].rearrange("(p t) d -> p t d", p=P),
            }
            xf = {}

            def dma_quarter(eng, name, qt, dtype=F32):
                t0 = 4 * qt
                buf = xin.tile(
                    [P, 4, D], dtype, tag=f"xf_{name}{qt}", name=f"xf_{name}{qt}"
                )
                xf[(name, qt)] = buf
                eng.dma_start(buf[:], srcs[name][:, t0 : t0 + 4, :])

            def dma_eighth(eng, name, et, dtype=F32):
                t0 = 2 * et
                buf = xin.tile(
                    [P, 2, D], dtype, tag=f"xe_{name}{et}", name=f"xe_{name}{et}"
                )
                xf[(name, "e", et)] = buf
                eng.dma_start(buf[:], srcs[name][:, t0 : t0 + 2, :])

            # deadline-ordered dispatch across the three queues
            dma_eighth(nc.sync, "k", 0)
            dma_quarter(nc.sync, "q", 1)
            dma_eighth(nc.sync, "k", 1)
            dma_quarter(nc.sync, "k", 1)

            dma_quarter(nc.scalar, "q", 0)
            # Wk/Wq fp32 on scalar after q0; cast to bf16 on DVE
            for name, ext in (("Wk", wk_ext), ("Wq", wq_ext)):
                wf = const.tile([P, D], F32, tag=f"wf_{name}", name=f"wf_{name}")
                nc.scalar.dma_start(wf[:], ext[:])
                w_f32[name] = wf
                wb = const.tile([P, D], BF16, tag=f"w_{name}", name=f"w_{name}")
                nc.vector.tensor_copy(out=wb[:], in_=wf[:])
                w_bf[name] = wb
            dma_quarter(nc.scalar, "k", 2)
            # vT via XBAR transpose once the scratch cast lands
            vT_sb = big.tile([P, L], BF16, tag="vT")
            nc.scalar.dma_start(vT_sb[:], vbf_dram[:], transpose=True)
            w_bf["Wv"] = wv_b
            dma_quarter(nc.gpsimd, "k", 3, BF16)
            dma_quarter(nc.gpsimd, "q", 2, BF16)
            dma_quarter(nc.gpsimd, "q", 3, BF16)

            # mask bias on Pool (frees DVE)
            mask_bias = const.tile([P, T], F32, tag="maskb")
            nc.gpsimd.tensor_scalar(
                mask_bias[:], mask_i[:], 10000.0, -10000.0, MULT, ADD
            )

            # exp table preload on ACT (after its DMA dispatches)
            dummy_exp = const.tile([P, 1], F32, tag="dummy")
            nc.scalar.activation(dummy_exp[:], ident_f[:, :1], EXP)

            # ---- PE warm-up fillers: keep HAM clock gate open ----
            def fillers(n):
                for _ in range(n):
                    wps = ps_o.tile([P, 1024], F32, tag="oT", name="warmps")
                    nc.tensor.matmul(
                        wps[:, :512], warm[:, :P], warm[:], start=True, stop=True
                    )

            # ---- persistent big tensors ----
            qT_h = [
                big.tile([P, 1024], BF16, tag=f"qT{h}", name=f"qT{h}")
                for h in range(2)
            ]
            kT_h = [
                big.tile([P, 1024], BF16, tag=f"kT{h}", name=f"kT{h}")
                for h in range(2)
            ]
            v_h = [
                big.tile([P, 1024], BF16, tag=f"v{h}", name=f"v{h}")
                for h in range(2)
            ]
            S_h = [
                big.tile([P, 1024], BF16, tag=f"S{h}", name=f"S{h}")
                for h in range(2)
            ]

            def xpose_quarter(name, qt):
                """PE transpose of 4 tiles -> bf16 SBUF [128d, 4, 128]."""
                buf = xf[(name, qt)]
                pst = psp.tile([P, 512], F32, tag="psp", name=f"tp_{name}{qt}")
                for c in range(4):
                    if buf.dtype == F32:
                        nc.tensor.transpose(
                            pst[:, c * P : (c + 1) * P], buf[:, c, :], ident_f[:]
                        )
                    else:
                        nc.tensor.matmul(
                            pst[:, c * P : (c + 1) * P],
                            buf[:, c, :],
                            ident[:],
                            start=True,
                            stop=True,
                        )
                xb = xt.tile([P, 4, P], BF16, tag="xT", name=f"xT_{name}{qt}")
                nc.vector.tensor_copy(
                    out=xb[:].rearrange("p a b -> p (a b)"), in_=pst[:]
                )
                return xb

            def proj_qk_quarter(xb, wname, dst_h, qt):
                pst = psp.tile([P, 512], F32, tag="psp", name=f"pj{wname}{qt}")
                nc.tensor.matmul(
                    pst[:],
                    w_bf[wname][:],
                    xb[:].rearrange("p a b -> p (a b)"),
                    start=True,
                    stop=True,
                )
                nc.vector.tensor_copy(
                    out=dst_h[:, (qt % 2) * 512 : (qt % 2) * 512 + 512],
                    in_=pst[:],
                )

            def proj_v_quarter(qt):
                vT_t = vT_sb[:].rearrange("d (p t) -> d t p", t=T)
                pst = psp.tile([P, 512], F32, tag="psp", name=f"pjv{qt}")
                for c in range(4):
                    nc.tensor.matmul(
                        pst[:, c * P : (c + 1) * P],
                        vT_t[:, 4 * qt + c, :],
                        w_bf["Wv"][:],
                        start=True,
                        stop=True,
                    )
                nc.vector.tensor_copy(
                    out=v_h[qt // 2][:, (qt % 2) * 512 : (qt % 2) * 512 + 512],
                    in_=pst[:],
                )

            def prep_eighth(name, et, wname, dst_h):
                buf = xf[(name, "e", et)]
                pst = psp.tile([P, 256], F32, tag="psp", name=f"te_{name}{et}")
                for c in range(2):
                    nc.tensor.transpose(
                        pst[:, c * P : (c + 1) * P], buf[:, c, :], ident_f[:]
                    )
                xb = xt.tile([P, 2, P], BF16, tag="xTe", name=f"xTe_{name}{et}")
                nc.vector.tensor_copy(
                    out=xb[:].rearrange("p a b -> p (a b)"), in_=pst[:]
                )
                pst2 = psp.tile([P, 256], F32, tag="psp", name=f"pe_{name}{et}")
                nc.tensor.matmul(
                    pst2[:],
                    w_bf[wname][:],
                    xb[:].rearrange("p a b -> p (a b)"),
                    start=True,
                    stop=True,
                )
                nc.vector.tensor_copy(
                    out=dst_h[:, et * 256 : et * 256 + 256], in_=pst2[:]
                )

            # ---- prefill: k tiles 0-3 (eighths), q.h0 (rest in-loop) ----
            fillers(3)
            prep_eighth("k", 0, "Wk", kT_h[0])
            xb_q0 = xpose_quarter("q", 0)
            proj_qk_quarter(xb_q0, "Wq", qT_h[0], 0)
            xb_q1 = xpose_quarter("q", 1)
            proj_qk_quarter(xb_q1, "Wq", qT_h[0], 1)
            prep_eighth("k", 1, "Wk", kT_h[0])

            # ---- main loop ----
            out_all = big.tile([P, T, D], BF16, tag="out_all")
            out_dst = out_ext[:].rearrange("(p t) d -> p t d", p=P)
            oT_hs = {}
            LAG = {0: 5, 1: 6}

            def emit_oT(h, j, a_list, cols=(0, 1)):
                for c in cols:
                    nc.tensor.matmul(
                        oT_hs[h][:, c * 512 : (c + 1) * 512],
                        v_h[j // 8][:, (j % 8) * D : (j % 8 + 1) * D],
                        a_list[j][:, c * 512 : (c + 1) * 512],
                        start=(j == 0),
                        stop=(j == T - 1),
                    )

            # in-loop prep schedule: (h, j) -> list of thunks.
            # Deadlines (global iter): k.Q1<4 k.Q2<8 k.Q3<12 v.Q0<5 v.Q1<9
            # v.Q2<13 q.Q2/3 proj<=14 v.Q3 proj<=15 (h0 flush at g16..g20)
            prep_xb = {}

            def mk_xpose(name, qt):
                def f():
                    prep_xb[(name, qt)] = xpose_quarter(name, qt)

                return f

            def mk_proj(name, qt):
                def f():
                    if name == "v":
                        proj_v_quarter(qt)
                        return
                    xb = prep_xb[(name, qt)]
                    if name == "k":
                        proj_qk_quarter(xb, "Wk", kT_h[qt // 2], qt)
                    else:
                        proj_qk_quarter(xb, "Wq", qT_h[qt // 2], qt)

                return f

            sched = {
                (0, 0): [mk_xpose("k", 1)],
                (0, 1): [mk_proj("k", 1)],
                (0, 2): [mk_proj("v", 0)],
                (0, 3): [mk_proj("v", 1)],
                (0, 4): [mk_xpose("k", 2)],
                (0, 5): [mk_proj("k", 2)],
                (0, 6): [mk_xpose("q", 2)],
                (0, 7): [mk_proj("q", 2)],
                (0, 8): [mk_xpose("k", 3)],
                (0, 9): [mk_proj("k", 3)],
                (0, 10): [mk_xpose("q", 3)],
                (0, 11): [mk_proj("q", 3)],
                (0, 12): [mk_proj("v", 2)],
                (0, 13): [mk_proj("v", 3)],
            }

            ones_col = const.tile([P, 1], BF16, tag="ones")
            nc.gpsimd.memset(ones_col[:], 1.0)
            denT0 = const.tile([P, 8], F32, tag="denT0")
            rT0 = const.tile([P, 8], F32, tag="rT0")
            oT_bf0 = big.tile([P, 1024], BF16, tag="oTb0")

            def epilogue_half0_step(step):
                """h0 epilogue spread over h1 iters j=5..10 (g21..g26)."""
                if step == 0:
                    dps = psp.tile([P, 8], F32, tag="psp", name="dps0")
                    for tt in range(8):
                        nc.tensor.matmul(
                            dps[:, tt : tt + 1],
                            S_h[0][:, tt * P : (tt + 1) * P],
                            ones_col[:],
                            start=True,
                            stop=True,
                        )
                    nc.vector.tensor_copy(out=denT0[:], in_=dps[:])
                    nc.vector.reciprocal(rT0[:], denT0[:])
                    nc.vector.tensor_copy(
                        out=oT_bf0[:, :512], in_=oT_hs[0][:, :512]
                    )
                elif step == 1:
                    nc.vector.tensor_copy(
                        out=oT_bf0[:, 512:], in_=oT_hs[0][:, 512:]
                    )
                elif step in (2, 3):
                    g = step - 2
                    tps = psp.tile([P, 512], F32, tag="psp", name=f"tps0_{g}")
                    for c in range(4):
                        tt = g * 4 + c
                        nc.tensor.matmul(
                            tps[:, c * P : (c + 1) * P],
                            oT_bf0[:, tt * P : (tt + 1) * P],
                            ident[:],
                            start=True,
                            stop=True,
                        )
                    for c in range(4):
                        tt = g * 4 + c
                        nc.vector.tensor_scalar_mul(
                            out_all[:, tt, :],
                            tps[:, c * P : (c + 1) * P],
                            rT0[:, tt : tt + 1],
                        )
                elif step == 4:
                    nc.sync.dma_start(out_dst[:, 0:4, :], out_all[:, 0:4, :])
                elif step == 5:
                    nc.sync.dma_start(out_dst[:, 4:8, :], out_all[:, 4:8, :])

            def s_accum(h, j, a):
                if h == 1 and j == 15:
                    return  # h1 tail computes den from S(..14) + colsums(a15)
                if j == 0:
                    nc.vector.tensor_copy(out=S_h[h][:], in_=a[:])
                else:
                    nc.vector.tensor_tensor(S_h[h][:], S_h[h][:], a[:], ADD)

            a_lists = {}
            for h in range(2):
                oT_hs[h] = ps_o.tile([P, 1024], F32, tag="oT", name=f"oT{h}")
                a_list = []
                a_lists[h] = a_list
                for j in range(T):
                    sps = ps.tile([P, 1024], F32, tag="ps", name=f"sT{h}_{j}")
                    for c in range(2):
                        nc.tensor.matmul(
                            sps[:, c * 512 : (c + 1) * 512],
                            kT_h[j // 8][:, (j % 8) * P : (j % 8 + 1) * P],
                            qT_h[h][:, c * 512 : (c + 1) * 512],
                            start=True,
                            stop=True,
                        )
                    a = att.tile([P, 1024], BF16, tag="aT", name=f"aT{h}_{j}")
                    nc.scalar.activation(
                        a[:],
                        sps[:],
                        EXP,
                        bias=mask_bias[:, j : j + 1],
                        scale=SCALE,
                    )
                    a_list.append(a)
                    for f in sched.get((h, j), ()):
                        f()
                    s_accum(h, j, a)
                    if h == 1 and j <= 4:
                        # flush h0's lagged oT (needs v tiles 11..15)
                        emit_oT(0, 11 + j, a_lists[0])
                    if h == 1 and 5 <= j <= 10:
                        epilogue_half0_step(j - 5)
                    if j >= LAG[h]:
                        emit_oT(h, j - LAG[h], a_list)
                    if h == 1 and 12 <= j <= 15:
                        # catch-up: double-emit so only jj=14,15 remain
                        emit_oT(h, j - LAG[h] + 4, a_list)

            # ---- tail: jj=14,15 remain for oT, then epilogue ----
            h = 1
            a_last = a_lists[1]
            # den part A: colsums of S(j<=14); runs while exp(a15) streams
            dps = psp.tile([P, 16], F32, tag="psp", name="dps1")
            for tt in range(8):
                nc.tensor.matmul(
                    dps[:, tt : tt + 1],
                    S_h[1][:, tt * P : (tt + 1) * P],
                    ones_col[:],
                    start=True,
                    stop=True,
                )
            emit_oT(h, 14, a_last)
            # den part B: colsums(a15) -- only waits on the last exp
            for tt in range(8):
                nc.tensor.matmul(
                    dps[:, 8 + tt : 9 + tt],
                    a_last[15][:, tt * P : (tt + 1) * P],
                    ones_col[:],
                    start=True,
                    stop=True,
                )
            emit_oT(h, 15, a_last)
            denT1a = const.tile([P, 8], F32, tag="denT1a")
            nc.vector.tensor_copy(out=denT1a[:], in_=dps[:, :8])
            denT1 = const.tile([P, 8], F32, tag="denT1")
            nc.vector.tensor_tensor(denT1[:], denT1a[:], dps[:, 8:16], ADD)
            rT1 = const.tile([P, 8], F32, tag="rT1")
            nc.vector.reciprocal(rT1[:], denT1[:])
            oT_bf1 = big.tile([P, 1024], BF16, tag="oTb1")
            for g in range(2):
                nc.vector.tensor_copy(
                    out=oT_bf1[:, g * 512 : (g + 1) * 512],
                    in_=oT_hs[1][:, g * 512 : (g + 1) * 512],
                )
                tps = psp.tile([P, 512], F32, tag="psp", name=f"tps1_{g}")
                for c in range(4):
                    tt = g * 4 + c
                    nc.tensor.matmul(
                        tps[:, c * P : (c + 1) * P],
                        oT_bf1[:, tt * P : (tt + 1) * P],
                        ident[:],
                        start=True,
                        stop=True,
                    )
                # scaled copies: ACT takes even tiles, DVE odd tiles
                for c in range(4):
                    tt = g * 4 + c
                    if c % 2 == 0:
                        nc.scalar.activation(
                            out_all[:, 8 + tt, :],
                            tps[:, c * P : (c + 1) * P],
                            COPY_FN,
                            scale=rT1[:, tt : tt + 1],
                        )
                    else:
                        nc.vector.tensor_scalar_mul(
                            out_all[:, 8 + tt, :],
                            tps[:, c * P : (c + 1) * P],
                            rT1[:, tt : tt + 1],
                        )
                        if g == 0 and c == 3:
                            nc.sync.dma_start(
                                out_dst[:, 8:12, :], out_all[:, 8:12, :]
                            )
                        elif g == 1 and c == 1:
                            nc.scalar.dma_start(
                                out_dst[:, 12:14, :], out_all[:, 12:14, :]
                            )
                        elif g == 1 and c == 3:
                            nc.sync.dma_start(
                                out_dst[:, 14:16, :], out_all[:, 14:16, :]
                            )

    nc.compile()
    return nc


_NC_CACHE = None


def _get_nc():
    global _NC_CACHE
    if _NC_CACHE is None:
        _NC_CACHE = build()
    return _NC_CACHE


def kernel(query, key, value, Wq, Wk, Wv, attention_mask):
    query = np.asarray(query, dtype=np.float32)
    key = np.asarray(key, dtype=np.float32)
    value = np.asarray(value, dtype=np.float32)
    Wq = np.asarray(Wq, dtype=np.float32)
    Wk = np.asarray(Wk, dtype=np.float32)
    Wv = np.asarray(Wv, dtype=np.float32)
    mask = np.asarray(attention_mask, dtype=np.int32).reshape(N_CORES, 1, L)

    nc = _get_nc()
    in_maps = [
        {
            "query": np.ascontiguousarray(query[b]),
            "key": np.ascontiguousarray(key[b]),
            "value": np.ascontiguousarray(value[b]),
            "Wq": Wq,
            "Wk": Wk,
            "Wv": Wv,
            "mask": np.ascontiguousarray(mask[b]),
        }
        for b in range(N_CORES)
    ]
    res = run_bass_kernel_spmd(nc, in_maps, core_ids=list(range(N_CORES)))
    out = np.stack(
        [np.asarray(res.results[b]["out"]) for b in range(N_CORES)], axis=0
    )
    return out.astype(np.float32)


if __name__ == "__main__":
    rng = np.random.default_rng(0)
    q = rng.standard_normal((N_CORES, L, D), dtype=np.float32)
    k = rng.standard_normal((N_CORES, L, D), dtype=np.float32)
    v = rng.standard_normal((N_CORES, L, D), dtype=np.float32)
    wq = rng.standard_normal((D, D), dtype=np.float32) * 0.08
    wk = rng.standard_normal((D, D), dtype=np.float32) * 0.08
    wv = rng.standard_normal((D, D), dtype=np.float32) * 0.08
    m = np.ones((N_CORES, 1, L), dtype=np.int32)
    out = kernel(
        query=q, key=k, value=v, Wq=wq, Wk=wk, Wv=wv, attention_mask=m
    )
    print(out.shape, out.dtype)


# revision 22
# speedup vs baseline: 1.0603x; 1.0603x over previous
"""Trainium2 Bass kernel for batched attention (B=8, Lq=Lk=2048, D=Dv=128).

Sharding: pure data parallel — batch element b runs on NeuronCore b.
Per-core algorithm (all big matmuls bf16 with fp32 PSUM accumulation):

  qT = Wq^T @ xq^T        [d, Lq]   (PE transposes raw tiles)
  kT = Wk^T @ xk^T        [d, Lk]
  v  = xv @ Wv            [Lk, dv]
  for each k-tile j (16 tiles of 128):
      sT_j = kT_j^T @ qT  [128k, Lq]   (scores, TRANSPOSED: k on partitions)
      aT_j = exp(sT_j * scale + mask_bias)   (ACT, psum->sbuf, bf16)
      S   += aT_j                 (running col-sum: DVE cols 0:640, Pool 640:)
      oT  += v_j^T @ aT_j [dv, Lq]      (PSUM accumulate across j)
  denT[:, t] = S_t^T @ ones             (per-q softmax denominators)
  out_t = (oT_t)^T * recip(denT_t)

Schedule: inputs stream in as quarters over all three DMA queues
(sync/scalar HWDGE fp32 + gpsimd SWDGE casting to bf16) dispatched at
program start in deadline order; prep (transpose+proj) for everything
past k.Q0/q.h0 is interleaved into the 32-iteration main loop; PE
fillers keep the HAM clock gate open until the first real transpose;
epilogues are pipelined and the final half double-emits lagged oT
matmuls so only 2 k-tiles remain to flush at the end.
"""

import sys

sys.path.insert(0, "/opt/trn_rl_repo")

import numpy as np

import concourse.bass as bass
import concourse.mybir as mybir
import concourse.tile as tile
from concourse import bacc
from concourse.bass_utils import run_bass_kernel_spmd
from concourse.masks import make_identity

P = 128
L = 2048
D = 128
T = L // P  # 16 tiles
F32 = mybir.dt.float32
I32 = mybir.dt.int32
BF16 = mybir.dt.bfloat16
SCALE = 1.0 / float(np.sqrt(128.0))
N_CORES = 8
SSPLIT = 640  # S-accumulation column split: DVE [0:640), Pool [640:)

ADD = mybir.AluOpType.add
MULT = mybir.AluOpType.mult
EXP = mybir.ActivationFunctionType.Exp
COPY_FN = mybir.ActivationFunctionType.Copy


def build():
    nc = bacc.Bacc("TRN2", target_bir_lowering=False, debug=False)

    q_ext = nc.declare_dram_parameter("query", [L, D], F32, isOutput=False)
    k_ext = nc.declare_dram_parameter("key", [L, D], F32, isOutput=False)
    v_ext = nc.declare_dram_parameter("value", [L, D], F32, isOutput=False)
    wq_ext = nc.declare_dram_parameter("Wq", [D, D], F32, isOutput=False)
    wk_ext = nc.declare_dram_parameter("Wk", [D, D], F32, isOutput=False)
    wv_ext = nc.declare_dram_parameter("Wv", [D, D], F32, isOutput=False)
    m_ext = nc.declare_dram_parameter("mask", [1, L], I32, isOutput=False)
    out_ext = nc.declare_dram_parameter("out", [L, D], BF16, isOutput=True)

    with tile.TileContext(nc) as tc:
        with (
            tc.tile_pool(name="const", bufs=1) as const,
            tc.tile_pool(name="big", bufs=1) as big,
            tc.tile_pool(name="xin", bufs=1) as xin,
            tc.tile_pool(name="xt", bufs=4) as xt,
            tc.tile_pool(name="att", bufs=16) as att,
            # 2 x [128,1024]f32 slots = 4 banks (scores rotation)
            tc.tile_pool(name="ps", bufs=2, space="PSUM") as ps,
            # 2 x [128,512]f32 slots = 2 banks (prep/den/out-transpose)
            tc.tile_pool(name="psp", bufs=2, space="PSUM") as psp,
            # 1 x [128,1024]f32 slot = 2 banks (oT accumulator + warmup)
            tc.tile_pool(name="ps_o", bufs=1, space="PSUM") as ps_o,
        ):
            # ---- gpsimd: warm tile first (unblocks PE), then DMAs ----
            warm = const.tile([P, 512], BF16, tag="warm")
            nc.gpsimd.memset(warm[:], 0.125)

            # identities first on Pool: the first transpose needs ident_f
            ident_f = const.tile([P, P], F32, tag="identf")
            make_identity(nc, ident_f[:])
            ident = const.tile([P, P], BF16, tag="ident")
            nc.gpsimd.tensor_copy(out=ident[:], in_=ident_f[:])

            w_bf = {}
            w_f32 = {}

            # mask bias: element k=(16*p+t) -> [p, t]; bias = (m-1)*1e4
            mask_i = const.tile([P, T], I32, tag="maski")
            nc.gpsimd.dma_start(
                mask_i[:], m_ext[:].rearrange("o (p t) -> p (o t)", p=P)
            )
            wv_b = const.tile([P, D], BF16, tag="w_Wv")
            nc.gpsimd.dma_start(wv_b[:], wv_ext[:])

            # v: cast fp32->bf16 into scratch DRAM (SWDGE), then XBAR
            # DMA-transpose the whole tensor into SBUF as vT [dv, Lk]
            vbf_dram = nc.dram_tensor("vbf_scratch", [L, D], BF16, kind="Internal")
            nc.gpsimd.dma_start(vbf_dram[:], v_ext[:])

            # ---- input quarters: [128, 4, 128], row = 16*p + t ----
            srcs = {
                "q": q_ext[:].rearrange("(p t) d -> p t d", p=P),
                "k": k_ext[:].rearrange("(p t) d -> p t d", p=P),
                "v": v_ext[:].rearrange("(p t) d -> p t d", p=P),
            }
            xf = {}

            def dma_quarter(eng, name, qt, dtype=F32):
                t0 = 4 * qt
                buf = xin.tile(
                    [P, 4, D], dtype, tag=f"xf_{name}{qt}", name=f"xf_{name}{qt}"
                )
                xf[(name, qt)] = buf
                eng.dma_start(buf[:], srcs[name][:, t0 : t0 + 4, :])

            def dma_eighth(eng, name, et, dtype=F32):
                t0 = 2 * et
                buf = xin.tile(
                    [P, 2, D], dtype, tag=f"xe_{name}{et}", name=f"xe_{name}{et}"
                )
                xf[(name, "e", et)] = buf
                eng.dma_start(buf[:], srcs[name][:, t0 : t0 + 2, :])

            # deadline-ordered dispatch across the three queues
            dma_eighth(nc.sync, "k", 0)
            dma_quarter(nc.sync, "q", 1)
            dma_eighth(nc.sync, "k", 1)
            dma_quarter(nc.sync, "k", 1)

            dma_quarter(nc.scalar, "q", 0)
            # Wk/Wq fp32 on scalar after q0; cast to bf16 on DVE
            for name, ext in (("Wk", wk_ext), ("Wq", wq_ext)):
                wf = const.tile([P, D], F32, tag=f"wf_{name}", name=f"wf_{name}")
                nc.scalar.dma_start(wf[:], ext[:])
                w_f32[name] = wf
                wb = const.tile([P, D], BF16, tag=f"w_{name}", name=f"w_{name}")
                nc.vector.tensor_copy(out=wb[:], in_=wf[:])
                w_bf[name] = wb
            dma_quarter(nc.scalar, "k", 2)
            # vT via XBAR transpose once the scratch cast lands
            vT_sb = big.tile([P, L], BF16, tag="vT")
            nc.scalar.dma_start(vT_sb[:], vbf_dram[:], transpose=True)
            w_bf["Wv"] = wv_b
            dma_quarter(nc.gpsimd, "k", 3, BF16)
            dma_quarter(nc.gpsimd, "q", 2, BF16)
            dma_quarter(nc.gpsimd, "q", 3, BF16)

            # mask bias on Pool (frees DVE)
            mask_bias = const.tile([P, T], F32, tag="maskb")
            nc.gpsimd.tensor_scalar(
                mask_bias[:], mask_i[:], 10000.0, -10000.0, MULT, ADD
            )

            # exp table preload on ACT (after its DMA dispatches)
            dummy_exp = const.tile([P, 1], F32, tag="dummy")
            nc.scalar.activation(dummy_exp[:], ident_f[:, :1], EXP)

            # ---- PE warm-up fillers: keep HAM clock gate open ----
            def fillers(n):
                for _ in range(n):
                    wps = ps_o.tile([P, 1024], F32, tag="oT", name="warmps")
                    nc.tensor.matmul(
                        wps[:, :512], warm[:, :P], warm[:], start=True, stop=True
                    )

            # ---- persistent big tensors ----
            qT_h = [
                big.tile([P, 1024], BF16, tag=f"qT{h}", name=f"qT{h}")
                for h in range(2)
            ]
            kT_h = [
                big.tile([P, 1024], BF16, tag=f"kT{h}", name=f"kT{h}")
                for h in range(2)
            ]
            v_h = [
                big.tile([P, 1024], BF16, tag=f"v{h}", name=f"v{h}")
                for h in range(2)
            ]
            S_h = [
                big.tile([P, 1024], BF16, tag=f"S{h}", name=f"S{h}")
                for h in range(2)
            ]

            def xpose_quarter(name, qt):
                """PE transpose of 4 tiles -> bf16 SBUF [128d, 4, 128]."""
                buf = xf[(name, qt)]
                pst = psp.tile([P, 512], F32, tag="psp", name=f"tp_{name}{qt}")
                for c in range(4):
                    if buf.dtype == F32:
                        nc.tensor.transpose(
                            pst[:, c * P : (c + 1) * P], buf[:, c, :], ident_f[:]
                        )
                    else:
                        nc.tensor.matmul(
                            pst[:, c * P : (c + 1) * P],
                            buf[:, c, :],
                            ident[:],
                            start=True,
                            stop=True,
                        )
                xb = xt.tile([P, 4, P], BF16, tag="xT", name=f"xT_{name}{qt}")
                nc.vector.tensor_copy(
                    out=xb[:].rearrange("p a b -> p (a b)"), in_=pst[:]
                )
                return xb

            def proj_qk_quarter(xb, wname, dst_h, qt):
                pst = psp.tile([P, 512], F32, tag="psp", name=f"pj{wname}{qt}")
                nc.tensor.matmul(
                    pst[:],
                    w_bf[wname][:],
                    xb[:].rearrange("p a b -> p (a b)"),
                    start=True,
                    stop=True,
                )
                nc.vector.tensor_copy(
                    out=dst_h[:, (qt % 2) * 512 : (qt % 2) * 512 + 512],
                    in_=pst[:],
                )

            def proj_v_quarter(qt):
                vT_t = vT_sb[:].rearrange("d (p t) -> d t p", t=T)
                pst = psp.tile([P, 512], F32, tag="psp", name=f"pjv{qt}")
                for c in range(4):
                    nc.tensor.matmul(
                        pst[:, c * P : (c + 1) * P],
                        vT_t[:, 4 * qt + c, :],
                        w_bf["Wv"][:],
                        start=True,
                        stop=True,
                    )
                nc.vector.tensor_copy(
                    out=v_h[qt // 2][:, (qt % 2) * 512 : (qt % 2) * 512 + 512],
                    in_=pst[:],
                )

            def prep_eighth(name, et, wname, dst_h):
                buf = xf[(name, "e", et)]
                pst = psp.tile([P, 256], F32, tag="psp", name=f"te_{name}{et}")
                for c in range(2):
                    nc.tensor.transpose(
                        pst[:, c * P : (c + 1) * P], buf[:, c, :], ident_f[:]
                    )
                xb = xt.tile([P, 2, P], BF16, tag="xTe", name=f"xTe_{name}{et}")
                nc.vector.tensor_copy(
                    out=xb[:].rearrange("p a b -> p (a b)"), in_=pst[:]
                )
                pst2 = psp.tile([P, 256], F32, tag="psp", name=f"pe_{name}{et}")
                nc.tensor.matmul(
                    pst2[:],
                    w_bf[wname][:],
                    xb[:].rearrange("p a b -> p (a b)"),
                    start=True,
                    stop=True,
                )
                nc.vector.tensor_copy(
                    out=dst_h[:, et * 256 : et * 256 + 256], in_=pst2[:]
                )

            # ---- prefill: k tiles 0-3 (eighths), q.h0 (rest in-loop) ----
            fillers(3)
            prep_eighth("k", 0, "Wk", kT_h[0])
            xb_q0 = xpose_quarter("q", 0)
            proj_qk_quarter(xb_q0, "Wq", qT_h[0], 0)
            xb_q1 = xpose_quarter("q", 1)
            proj_qk_quarter(xb_q1, "Wq", qT_h[0], 1)
            prep_eighth("k", 1, "Wk", kT_h[0])

            # ---- main loop ----
            out_all = big.tile([P, T, D], BF16, tag="out_all")
            out_dst = out_ext[:].rearrange("(p t) d -> p t d", p=P)
            oT_hs = {}
            LAG = {0: 5, 1: 6}

            def emit_oT(h, j, a_list, cols=(0, 1)):
                for c in cols:
                    nc.tensor.matmul(
                        oT_hs[h][:, c * 512 : (c + 1) * 512],
                        v_h[j // 8][:, (j % 8) * D : (j % 8 + 1) * D],
                        a_list[j][:, c * 512 : (c + 1) * 512],
                        start=(j == 0),
                        stop=(j == T - 1),
                    )

            # in-loop prep schedule: (h, j) -> list of thunks.
            # Deadlines (global iter): k.Q1<4 k.Q2<8 k.Q3<12 v.Q0<5 v.Q1<9
            # v.Q2<13 q.Q2/3 proj<=14 v.Q3 proj<=15 (h0 flush at g16..g20)
            prep_xb = {}

            def mk_xpose(name, qt):
                def f():
                    prep_xb[(name, qt)] = xpose_quarter(name, qt)

                return f

            def mk_proj(name, qt):
                def f():
                    if name == "v":
                        proj_v_quarter(qt)
                        return
                    xb = prep_xb[(name, qt)]
                    if name == "k":
                        proj_qk_quarter(xb, "Wk", kT_h[qt // 2], qt)
                    else:
                        proj_qk_quarter(xb, "Wq", qT_h[qt // 2], qt)

                return f

            sched = {
                (0, 0): [mk_xpose("k", 1)],
                (0, 1): [mk_proj("k", 1)],
                (0, 2): [mk_proj("v", 0)],
                (0, 3): [mk_proj("v", 1)],
                (0, 4): [mk_xpose("k", 2)],
                (0, 5): [mk_proj("k", 2)],
                (0, 6): [mk_xpose("q", 2)],
                (0, 7): [mk_proj("q", 2)],
                (0, 8): [mk_xpose("k", 3)],
                (0, 9): [mk_proj("k", 3)],
                (0, 10): [mk_xpose("q", 3)],
                (0, 11): [mk_proj("q", 3)],
                (0, 12): [mk_proj("v", 2)],
                (0, 13): [mk_proj("v", 3)],
            }

            ones_col = const.tile([P, 1], BF16, tag="ones")
            nc.gpsimd.memset(ones_col[:], 1.0)
            denT0 = const.tile([P, 8], F32, tag="denT0")
            rT0 = const.tile([P, 8], F32, tag="rT0")
            oT_bf0 = big.tile([P, 1024], BF16, tag="oTb0")

            def epilogue_half0_step(step):
                """h0 epilogue spread over h1 iters j=5..10 (g21..g26)."""
                if step == 0:
                    dps = psp.tile([P, 8], F32, tag="psp", name="dps0")
                    for tt in range(8):
                        nc.tensor.matmul(
                            dps[:, tt : tt + 1],
                            S_h[0][:, tt * P : (tt + 1) * P],
                            ones_col[:],
                            start=True,
                            stop=True,
                        )
                    nc.vector.tensor_copy(out=denT0[:], in_=dps[:])
                    nc.vector.reciprocal(rT0[:], denT0[:])
                    nc.vector.tensor_copy(
                        out=oT_bf0[:, :512], in_=oT_hs[0][:, :512]
                    )
                elif step == 1:
                    nc.vector.tensor_copy(
                        out=oT_bf0[:, 512:], in_=oT_hs[0][:, 512:]
                    )
                elif step in (2, 3):
                    g = step - 2
                    tps = psp.tile([P, 512], F32, tag="psp", name=f"tps0_{g}")
                    for c in range(4):
                        tt = g * 4 + c
                        nc.tensor.matmul(
                            tps[:, c * P : (c + 1) * P],
                            oT_bf0[:, tt * P : (tt + 1) * P],
                            ident[:],
                            start=True,
                            stop=True,
                        )
                    for c in range(4):
                        tt = g * 4 + c
                        nc.vector.tensor_scalar_mul(
                            out_all[:, tt, :],
                            tps[:, c * P : (c + 1) * P],
                            rT0[:, tt : tt + 1],
                        )
                elif step == 4:
                    nc.sync.dma_start(out_dst[:, 0:4, :], out_all[:, 0:4, :])
                elif step == 5:
                    nc.sync.dma_start(out_dst[:, 4:8, :], out_all[:, 4:8, :])

            def s_accum(h, j, a):
                if h == 1 and j == 15:
                    return  # h1 tail computes den from S(..14) + colsums(a15)
                if j == 0:
                    nc.vector.tensor_copy(out=S_h[h][:], in_=a[:])
                else:
                    nc.vector.tensor_tensor(S_h[h][:], S_h[h][:], a[:], ADD)

            a_lists = {}
            for h in range(2):
                oT_hs[h] = ps_o.tile([P, 1024], F32, tag="oT", name=f"oT{h}")
                a_list = []
                a_lists[h] = a_list
                for j in range(T):
                    sps = ps.tile([P, 1024], F32, tag="ps", name=f"sT{h}_{j}")
                    for c in range(2):
                        nc.tensor.matmul(
                            sps[:, c * 512 : (c + 1) * 512],
                            kT_h[j // 8][:, (j % 8) * P : (j % 8 + 1) * P],
                            qT_h[h][:, c * 512 : (c + 1) * 512],
                            start=True,
                            stop=True,
                        )
                    a = att.tile([P, 1024], BF16, tag="aT", name=f"aT{h}_{j}")
                    nc.scalar.activation(
                        a[:],
                        sps[:],
                        EXP,
                        bias=mask_bias[:, j : j + 1],
                        scale=SCALE,
                    )
                    a_list.append(a)
                    for f in sched.get((h, j), ()):
                        f()
                    s_accum(h, j, a)
                    if h == 1 and j <= 4:
                        # flush h0's lagged oT (needs v tiles 11..15)
                        emit_oT(0, 11 + j, a_lists[0])
                    if h == 1 and 5 <= j <= 10:
                        epilogue_half0_step(j - 5)
                    if j >= LAG[h]:
                        emit_oT(h, j - LAG[h], a_list)
                    if h == 1 and 12 <= j <= 15:
                        # catch-up: double-emit so only jj=14,15 remain
                        emit_oT(h, j - LAG[h] + 4, a_list)

            # ---- tail: jj=14,15 remain for oT, then epilogue ----
            h = 1
            a_last = a_lists[1]
            # den part A: colsums of S(j<=14); runs while exp(a15) streams
            dps = psp.tile([P, 16], F32, tag="psp", name="dps1")
            for tt in range(8):
                nc.tensor.matmul(
                    dps[:, tt : tt + 1],
                    S_h[1][:, tt * P : (tt + 1) * P],
                    ones_col[:],
                    start=True,
                    stop=True,
                )
            emit_oT(h, 14, a_last)
            # den part B: colsums(a15) -- only waits on the last exp
            for tt in range(8):
                nc.tensor.matmul(
                    dps[:, 8 + tt : 9 + tt],
                    a_last[15][:, tt * P : (tt + 1) * P],
                    ones_col[:],
                    start=True,
                    stop=True,
                )
            emit_oT(h, 15, a_last)
            denT1a = const.tile([P, 8], F32, tag="denT1a")
            nc.vector.tensor_copy(out=denT1a[:], in_=dps[:, :8])
            denT1 = const.tile([P, 8], F32, tag="denT1")
            nc.vector.tensor_tensor(denT1[:], denT1a[:], dps[:, 8:16], ADD)
            rT1 = const.tile([P, 8], F32, tag="rT1")
            nc.vector.reciprocal(rT1[:], denT1[:])
            oT_bf1 = big.tile([P, 1024], BF16, tag="oTb1")
            for g in range(2):
                nc.vector.tensor_copy(
                    out=oT_bf1[:, g * 512 : (g + 1) * 512],
                    in_=oT_hs[1][:, g * 512 : (g + 1) * 512],
                )
                tps = psp.tile([P, 512], F32, tag="psp", name=f"tps1_{g}")
                for c in range(4):
                    tt = g * 4 + c
                    nc.tensor.matmul(
                        tps[:, c * P : (c + 1) * P],
                        oT_bf1[:, tt * P : (tt + 1) * P],
                        ident[:],
                        start=True,
                        stop=True,
                    )
                # scaled copies: ACT takes even tiles, DVE odd tiles
                for c in range(4):
                    tt = g * 4 + c
                    if c % 2 == 0:
                        nc.scalar.activation(
                            out_all[:, 8 + tt, :],
                            tps[:, c * P : (c + 1) * P],
                            COPY_FN,
                            scale=rT1[:, tt : tt + 1],
                        )
                    else:
                        nc.vector.tensor_scalar_mul(
                            out_all[:, 8 + tt, :],
                            tps[:, c * P : (c + 1) * P],
                            rT1[:, tt : tt + 1],
                        )
                        a0 = 8 + tt - 1
                        nc.sync.dma_start(
                            out_dst[:, a0 : a0 + 2, :],
                            out_all[:, a0 : a0 + 2, :],
                        )

    nc.compile()
    return nc


_NC_CACHE = None


def _get_nc():
    global _NC_CACHE
    if _NC_CACHE is None:
        _NC_CACHE = build()
    return _NC_CACHE


def kernel(query, key, value, Wq, Wk, Wv, attention_mask):
    query = np.asarray(query, dtype=np.float32)
    key = np.asarray(key, dtype=np.float32)
    value = np.asarray(value, dtype=np.float32)
    Wq = np.asarray(Wq, dtype=np.float32)
    Wk = np.asarray(Wk, dtype=np.float32)
    Wv = np.asarray(Wv, dtype=np.float32)
    mask = np.asarray(attention_mask, dtype=np.int32).reshape(N_CORES, 1, L)

    nc = _get_nc()
    in_maps = [
        {
            "query": np.ascontiguousarray(query[b]),
            "key": np.ascontiguousarray(key[b]),
            "value": np.ascontiguousarray(value[b]),
            "Wq": Wq,
            "Wk": Wk,
            "Wv": Wv,
            "mask": np.ascontiguousarray(mask[b]),
        }
        for b in range(N_CORES)
    ]
    res = run_bass_kernel_spmd(nc, in_maps, core_ids=list(range(N_CORES)))
    out = np.stack(
        [np.asarray(res.results[b]["out"]) for b in range(N_CORES)], axis=0
    )
    return out.astype(np.float32)


if __name__ == "__main__":
    rng = np.random.default_rng(0)
    q = rng.standard_normal((N_CORES, L, D), dtype=np.float32)
    k = rng.standard_normal((N_CORES, L, D), dtype=np.float32)
    v = rng.standard_normal((N_CORES, L, D), dtype=np.float32)
    wq = rng.standard_normal((D, D), dtype=np.float32) * 0.08
    wk = rng.standard_normal((D, D), dtype=np.float32) * 0.08
    wv = rng.standard_normal((D, D), dtype=np.float32) * 0.08
    m = np.ones((N_CORES, 1, L), dtype=np.int32)
    out = kernel(
        query=q, key=k, value=v, Wq=wq, Wk=wk, Wv=wv, attention_mask=m
    )
    print(out.shape, out.dtype)
